# revision 1
# baseline (speedup 1.0000x reference)
"""AVWGCN (adaptive-vertex-weight GCN) Trainium2 kernel.

Math (per batch b, node n):
    S = E @ E.T                       [N, N]   (symmetric)
    Mexp = exp(relu(S))               [N, N]   (symmetric)
    r = 1 / rowsum(Mexp)              [N]
    A = diag(r) @ Mexp                (= softmax(relu(S), axis=1))
    z0 = x ;  z1 = diag(r) Mexp z0 ;  z2 = 2 diag(r) Mexp z1 - z0
    y[n, d, o]  = sum_{k,i} z_k[n, i] * Wp[d, k, i, o]
    out[b,n,o]  = sum_d E[n,d] * y[n,d,o] + (E @ bp)[n,o]

Sharding: data-parallel over batch B=64 across 8 cores (B_local=8).
Tricks:
  * Mexp symmetry -> same SBUF tile serves as lhsT for Mexp @ V.
  * z2 = u2' - x with u2' = 2 diag(r) Mexp z1; the "- x" is folded into
    the weights: x (W0 - W2) + z1 W1 + u2' W2, so x dies after phase 3.
  * z transposes (for the phase-5 contraction) stay in SBUF; z1/z2 are
    stitched into one K=128 stationary tile per batch.
"""
import sys

if "/opt/trn_rl_repo" not in sys.path:
    sys.path.insert(0, "/opt/trn_rl_repo")

import numpy as np
import bass_rust
import concourse.bass as bass
import concourse.mybir as mybir
from concourse import tile
from concourse.vector_clock import ScopedClock
from concourse.bass_utils import run_bass_kernel_spmd

F32 = mybir.dt.float32
F32R = mybir.dt.float32r
BF16 = mybir.dt.bfloat16
ALU = mybir.AluOpType
AF = mybir.ActivationFunctionType

B, N, CI, CO, KCH, D = 64, 2048, 64, 64, 3, 10
NCORES = 8
BL = B // NCORES          # 8 local batches
BC = BL * CI              # 512
NT = N // 128             # 16 node chunks
DO = D * CO               # 640


# ---------------------------------------------------------------- env patches
def _patched_drain_and_barrier(self, tick_clock, wait_clock):
    """Tail drain: walrus here allows only one sync-wait per instruction, so
    put each wait on its own sync nop before the drain."""
    nop_inst = self.nc.sync.nop(nofuse=True, hint="tile_tail_wait")
    wait_clock.add_sem_waits(nop_inst.ins,
                             ScopedClock({None: tick_clock.global_clock}))
    si = nop_inst.ins.sync_info
    waits = list(si.on_wait) if si is not None else []
    if len(waits) > 1:
        nop_inst.ins.sync_info = bass_rust.SyncInfo(
            on_wait=waits[:1], on_update=list(si.on_update))
        for w in waits[1:]:
            extra = self.nc.sync.nop(nofuse=True, hint="tile_tail_wait_x")
            extra.ins.sync_info = bass_rust.SyncInfo(on_wait=[w], on_update=[])
    self.nc.sync.drain()
    self.nc.all_engine_barrier()
    assert self.sems is not None
    popped = self.nc._tile_sem_poison_stack.pop()
    assert popped is self._sem_poison
    self.nc.clear_and_free_semaphores(list(self.sems.allocated().values()))
    self.nc.all_engine_barrier()


tile.TileContext._drain_and_barrier = _patched_drain_and_barrier


def split_multi_waits(nc):
    """Hoist extra sync-waits onto same-engine NoOps (1-wait walrus cap)."""
    for f in nc.m.functions:
        for bb in f.blocks:
            new = []
            for inst in bb.instructions:
                si = inst.sync_info
                if si is not None and len(si.on_wait) > 1:
                    waits = list(si.on_wait)
                    for w in waits[:-1]:
                        nop = mybir.InstNoOp(
                            name=nc.get_next_instruction_name(), ins=[], outs=[])
                        nop.engine = inst.engine
                        nop.sync_info = bass_rust.SyncInfo(on_wait=[w], on_update=[])
                        new.append(nop)
                    inst.sync_info = bass_rust.SyncInfo(
                        on_wait=[waits[-1]], on_update=list(si.on_update))
                new.append(inst)
            bb.instructions = new


# ---------------------------------------------------------------- kernel body
def build_nc():
    nc = bass.Bass()

    x_l = nc.dram_tensor("x_l", [BL, N, CI], F32, kind="ExternalInput")
    emb = nc.dram_tensor("emb", [N, D], F32, kind="ExternalInput")
    wp = nc.dram_tensor("wp", [D, KCH, CI, CO], F32, kind="ExternalInput")
    bp = nc.dram_tensor("bp", [D, CO], F32, kind="ExternalInput")
    ident = nc.dram_tensor("ident", [128, 128], F32, kind="ExternalInput")
    out_l = nc.dram_tensor("out_l", [BL, N, CI], F32, kind="ExternalOutput")

    with tile.TileContext(nc) as tc:
        with (
            tc.tile_pool(name="dram", bufs=1, space="DRAM") as dpool,
            tc.tile_pool(name="const", bufs=1) as cpool,
            tc.tile_pool(name="zt", bufs=1) as ztpool,
            tc.tile_pool(name="stage", bufs=1) as stpool,
            tc.tile_pool(name="mcol", bufs=3) as mcpool,
        ):
            mexp_d = dpool.tile([N, N], F32R, name="mexp_d")
            z0t_d = dpool.tile([BC, N], F32R, name="z0t_d")

            e_sb = cpool.tile([128, NT * D], F32)       # [p, nt*10+d]
            bias_sb = cpool.tile([128, NT * CO], F32)   # [p, nt*64+o]
            bp_sb = cpool.tile([D, CO], F32)
            id_r = cpool.tile([128, 128], F32R)
            id_f = cpool.tile([128, 128], F32)
            rowsum = cpool.tile([128, NT], F32)
            rinv = cpool.tile([128, NT], F32)
            r2 = cpool.tile([128, NT], F32)
            # weights: wr12 = [W1 ; W2] stacked, wr0m2 = (W0 - W2) duplicated
            wr12 = cpool.tile([128, DO], F32R)
            wr0m2 = cpool.tile([128, DO], F32R)

            # z1T/z2T stitched per local batch, resident in SBUF
            z12t = [ztpool.tile([128, N], F32R, name=f"z12t{b}", tag=f"z12t{b}")
                    for b in range(BL)]

            with (
                tc.tile_pool(name="bigz", bufs=1) as bzpool,
                tc.tile_pool(name="pstr", bufs=3, space="PSUM") as pstr,
            ):
                # 4 big tiles: phase-1 Mexp rows (f32r), then z1 storage
                bigz = [bzpool.tile([128, N], F32R, name=f"bigz{j}",
                                    tag=f"bigz{j}") for j in range(4)]

                def z1v(t):
                    return bigz[t // 4][:, (t % 4) * BC:(t % 4 + 1) * BC]

                # ---- constants / embeddings -------------------------------
                nc.sync.dma_start(id_f[:], ident[:])
                nc.vector.tensor_copy(id_r[:], id_f[:])
                nc.sync.dma_start(
                    e_sb[:].rearrange("p (nt d) -> p nt d", nt=NT),
                    emb[:].rearrange("(nt p) d -> p nt d", p=128))
                et4 = cpool.tile([64, N], F32, name="et4")
                for nt in range(NT):
                    pt = pstr.tile([128, 128], F32, tag="ptr")
                    nc.tensor.transpose(pt[0:D, 0:128],
                                        e_sb[:, nt * D:(nt + 1) * D], id_f[:])
                    nc.scalar.copy(et4[0:D, nt * 128:(nt + 1) * 128],
                                   pt[0:D, 0:128])

                nc.sync.dma_start(et4[32:32 + D, :], et4[0:D, :])

                nc.sync.dma_start(bp_sb[:], bp[:])
                for nt in range(NT):
                    pb = pstr.tile([128, 128], F32, tag="ptr")
                    nc.tensor.matmul(pb[:, 0:CO],
                                     et4[0:D, nt * 128:(nt + 1) * 128],
                                     bp_sb[:], start=True, stop=True)
                    nc.scalar.copy(bias_sb[:, nt * CO:(nt + 1) * CO], pb[:, 0:CO])

                # weights: load W0,W1,W2 as [i, (d,o)]
                wtmp = []
                for k in range(KCH):
                    wf = stpool.tile([64, DO], F32, tag=f"wload{k}")
                    nc.sync.dma_start(
                        wf[:].rearrange("p (d o) -> p d o", d=D),
                        wp[:, k, :, :].transpose([1, 0, 2]))
                    wtmp.append(wf)
                nc.vector.tensor_copy(wr12[0:64, :], wtmp[1][:])
                nc.vector.tensor_copy(wr12[64:128, :], wtmp[2][:])
                w0m2 = stpool.tile([64, DO], F32, tag="w0m2")
                nc.vector.tensor_sub(w0m2[:], wtmp[0][:], wtmp[2][:])
                nc.vector.tensor_copy(wr0m2[0:64, :], w0m2[:])
                nc.vector.tensor_copy(wr0m2[64:128, :], w0m2[:])

                # ---- phase 1: Mexp = exp(relu(E E^T)), rowsums ------------
                with (
                    tc.tile_pool(name="ps1", bufs=2, space="PSUM") as ps1,
                    tc.tile_pool(name="mfp", bufs=2) as mfpool,
                ):
                    for nt in range(NT):
                        m_f = mfpool.tile([128, N], F32, tag="m_f")
                        for half in range(2):
                            ps = ps1.tile([128, 1024], F32)
                            for j in range(2):
                                c0 = half * 1024 + j * 512
                                rb = 32 * j
                                nc.tensor.matmul(ps[:, j * 512:(j + 1) * 512],
                                                 et4[rb:rb + D,
                                                     nt * 128:(nt + 1) * 128],
                                                 et4[rb:rb + D, c0:c0 + 512],
                                                 start=True, stop=True,
                                                 tile_position=(rb, 0))
                            nc.vector.tensor_scalar_max(
                                m_f[:, half * 1024:(half + 1) * 1024], ps[:], 0.0)
                        m_r = bigz[nt % 4][:]
                        nc.scalar.activation(m_r, m_f[:], AF.Exp,
                                             accum_out=rowsum[:, nt:nt + 1])
                        eng = nc.sync if nt % 2 == 0 else nc.gpsimd
                        eng.dma_start(mexp_d[nt * 128:(nt + 1) * 128, :], m_r)

                # ---- phase 2 (early): load X as [m, (b,c)] ----------------
                xpool = tc.tile_pool(name="xp", bufs=1)
                xpool_cm = xpool.__enter__()
                x_t = [xpool_cm.tile([128, BC], F32R, name=f"x{t}",
                                     tag=f"x{t}") for t in range(NT)]
                x_r = [x_t[t][:] for t in range(NT)]
                for mt in range(NT):
                    (nc.sync if mt % 2 == 0 else nc.gpsimd).dma_start(
                        x_t[mt][:].rearrange("p (b c) -> p b c", b=BL),
                        x_l[:, mt * 128:(mt + 1) * 128, :]
                        .transpose([1, 0, 2]).bitcast(F32R))

                nc.vector.reciprocal(rinv[:], rowsum[:])
                nc.vector.tensor_scalar_mul(r2[:], rinv[:], 2.0)

                # ---- phase 3: z1 = diag(r) Mexp X ; z0T -------------------
                if True:
                    with tc.tile_pool(name="psmm3", bufs=3,
                                      space="PSUM") as psmm:
                        for nt in range(NT):
                            mcol = mcpool.tile([128, N], F32R, tag="mcol")
                            for q in range(4):
                                (nc.sync if (nt + q) % 2 == 0
                                 else nc.gpsimd).dma_start(
                                    mcol[:, q * 512:(q + 1) * 512]
                                    .rearrange("p (mt c) -> p mt c", mt=4),
                                    mexp_d[q * 512:(q + 1) * 512,
                                           nt * 128:(nt + 1) * 128]
                                    .rearrange("(mt p) c -> p mt c", p=128))
                            ps = psmm.tile([128, BC], F32)
                            for mt in range(NT):
                                nc.tensor.matmul(
                                    ps[:], mcol[:, mt * 128:(mt + 1) * 128],
                                    x_r[mt], start=(mt == 0),
                                    stop=(mt == NT - 1))
                            nc.scalar.activation(z1v(nt), ps[:], AF.Copy,
                                                 scale=rinv[:, nt:nt + 1])
                            # z0T: transpose x b-pair blocks, one DMA per nt
                            zst = stpool.tile([128, BC], F32R, tag="zst",
                                              bufs=3)
                            for w in range(BL // 2):
                                ptr = pstr.tile([128, 128], F32R, tag="ptr")
                                nc.tensor.transpose(
                                    ptr[:],
                                    x_r[nt][:, w * 128:(w + 1) * 128], id_r[:])
                                if w % 2 == 0:
                                    nc.scalar.copy(
                                        zst[:, w * 128:(w + 1) * 128], ptr[:])
                                else:
                                    nc.vector.tensor_copy(
                                        zst[:, w * 128:(w + 1) * 128], ptr[:])
                            nc.gpsimd.dma_start(
                                z0t_d[:, nt * 128:(nt + 1) * 128]
                                .rearrange("(w p) c -> p w c", p=128),
                                zst[:].rearrange("p (w c) -> p w c", w=4))

                xpool.__exit__(None, None, None)

                # ---- phase 4: u2' = 2 diag(r) Mexp z1 ; z1T/z2T stitch ----
                with tc.tile_pool(name="psmm4", bufs=3, space="PSUM") as psmm:
                    for nt in range(NT):
                        mcol = mcpool.tile([128, N], F32R, tag="mcol")
                        (nc.sync if nt % 2 == 0 else nc.gpsimd).dma_start(
                            mcol[:].rearrange("p (mt c) -> p mt c", mt=NT),
                            mexp_d[:, nt * 128:(nt + 1) * 128]
                            .rearrange("(mt p) c -> p mt c", p=128))
                        ps = psmm.tile([128, BC], F32)
                        for mt in range(NT):
                            nc.tensor.matmul(
                                ps[:], mcol[:, mt * 128:(mt + 1) * 128],
                                z1v(mt), start=(mt == 0), stop=(mt == NT - 1))
                        # interleave (z1_b | u2'_b) then one transpose per b:
                        # out partitions 0:64 = z1_b^T, 64:128 = u2'_b^T
                        zc = stpool.tile([128, BL * 128], F32R, tag="zc",
                                         bufs=4)
                        zcv = zc[:].rearrange("p (b c) -> p b c", b=BL)
                        nc.scalar.copy(
                            zcv[:, :, 0:64],
                            z1v(nt).rearrange("p (b c) -> p b c", b=BL))
                        nc.scalar.activation(
                            zcv[:, :, 64:128],
                            ps[:].rearrange("p (b c) -> p b c", b=BL),
                            AF.Copy, scale=r2[:, nt:nt + 1])
                        for b in range(BL):
                            ptr = pstr.tile([128, 128], F32R, tag="ptr")
                            nc.tensor.transpose(
                                ptr[:], zc[:, b * 128:(b + 1) * 128], id_r[:])
                            if b % 2 == 0:
                                nc.scalar.copy(
                                    z12t[b][:, nt * 128:(nt + 1) * 128], ptr[:])
                            else:
                                nc.vector.tensor_copy(
                                    z12t[b][:, nt * 128:(nt + 1) * 128], ptr[:])

            # ---- phase 5: y = zT . W ; out = sum_d E_d * y_d + bias -------
            with (
                tc.tile_pool(name="psy", bufs=2, space="PSUM") as psy,
                tc.tile_pool(name="slab", bufs=3) as slpool,
                tc.tile_pool(name="accp", bufs=2) as accpool,
            ):
                for nt in range(NT):
                    sl0 = slpool.tile([128, BC], F32R, tag="sl0")
                    nc.sync.dma_start(
                        sl0[:].rearrange("p (w c) -> p w c", w=4),
                        z0t_d[:, nt * 128:(nt + 1) * 128]
                        .rearrange("(w p) c -> p w c", p=128))
                    acc = accpool.tile([128, BC], F32, tag="acc", bufs=3)
                    accv = acc[:].rearrange("p (q o) -> p q o", q=BL)
                    bsl = bias_sb[:, nt * CO:(nt + 1) * CO]
                    bsrc = bass.AP(bsl.tensor, bsl.offset,
                                   [list(bsl.ap[0]), [0, BL], [1, CO]])
                    ytmp = accpool.tile([128, BL * DO], F32, tag="ytmp")
                    ytv = ytmp[:].rearrange("p (q f) -> p q f", q=BL)
                    nwin = slice(nt * 128, (nt + 1) * 128)
                    for bp2 in range(BL // 2):          # b pairs
                        ps = psy.tile([128, 2048], F32)
                        for bh in range(2):
                            b = bp2 * 2 + bh
                            po = bh * 1024
                            hp = (b % 2) * 64
                            wc = (b // 2) * 128
                            # 320+320 free split: f32r needs >=256-wide
                            # to run at 1 cyc/row; psum offsets keep each
                            # 320-wide write inside one bank
                            for w0, f0 in ((0, 0), (512, 320)):
                                nc.tensor.matmul(
                                    ps[:, po + w0:po + w0 + 320],
                                    z12t[b][:, nwin], wr12[:, f0:f0 + 320],
                                    start=True, stop=False)
                                nc.tensor.matmul(
                                    ps[:, po + w0:po + w0 + 320],
                                    sl0[hp:hp + 64, wc:wc + 128],
                                    wr0m2[hp:hp + 64, f0:f0 + 320],
                                    start=False, stop=True)
                        # evacuate this pair's y to SBUF (split ACT/DVE)
                        psv = (ps[:]
                               .rearrange("p (b h f) -> p b h f", b=2, h=2)
                               [:, :, :, 0:320])
                        nc.scalar.copy(
                            ytv[:, bp2 * 2:bp2 * 2 + 2, :]
                            .rearrange("p q (h f) -> p q h f", h=2), psv)
                    # d-contraction: one strided op per d over all 8 b;
                    # d=0 adds the (broadcast) bias instead of acc
                    for d in range(D):
                        nc.vector.scalar_tensor_tensor(
                            accv, ytv[:, :, d * CO:(d + 1) * CO],
                            e_sb[:, nt * D + d:nt * D + d + 1],
                            bsrc if d == 0 else accv,
                            op0=ALU.mult, op1=ALU.add)
                    nc.gpsimd.dma_start(
                        out_l[:, nt * 128:(nt + 1) * 128, :].transpose([1, 0, 2]),
                        accv)

    split_multi_waits(nc)
    return nc


_NC_CACHE = None


def get_nc():
    global _NC_CACHE
    if _NC_CACHE is None:
        _NC_CACHE = build_nc()
    return _NC_CACHE


def make_in_maps(inputs):
    x = np.ascontiguousarray(np.asarray(inputs["x"], dtype=np.float32))
    emb = np.ascontiguousarray(np.asarray(inputs["node_embeddings"],
                                          dtype=np.float32))
    wpa = np.ascontiguousarray(np.asarray(inputs["weights_pool"],
                                          dtype=np.float32))
    bpa = np.ascontiguousarray(np.asarray(inputs["bias_pool"],
                                          dtype=np.float32))
    ident = np.eye(128, dtype=np.float32)
    return [dict(x_l=x[c * BL:(c + 1) * BL], emb=emb, wp=wpa, bp=bpa,
                 ident=ident) for c in range(NCORES)]


def kernel(**inputs) -> np.ndarray:
    nc = get_nc()
    res = run_bass_kernel_spmd(nc, make_in_maps(inputs), list(range(NCORES)))
    out = np.concatenate([res.results[c]["out_l"] for c in range(NCORES)],
                         axis=0)
    return out.astype(np.float32)

